# revision 1
# baseline (speedup 1.0000x reference)
"""HGATConv (4-head graph attention, N=4096, F=512) on 8 Trainium2 NeuronCores.

Sharding: node rows split across 8 cores (512 rows each). Each core:
  - computes h = x @ W and S = x @ (W@A) for its own rows,
  - AllGathers [h | S] so every core has all nodes' features/scores,
  - computes its (H, 512, 4096) attention rows in transposed layout
    [j=partitions, i=free] so the weighted sum runs directly on the PE,
  - writes its 512 output rows.

Score math avoids a 2-pass activation via the identity
  exp(leakyrelu(s)) = max(exp(s), exp(s/5)),  s = si[i] + sj[j]
where exp(s) comes from one ACT pass (per-partition bias = sj) and
exp(s/5) = exp(si/5) * exp(sj/5) is a rank-1 DVE tensor_scalar product.
Row sums ride along as a ones-column appended to the rhs (129-wide matmuls).
"""

import sys
import numpy as np

if "/opt/trn_rl_repo" not in sys.path:
    sys.path.insert(0, "/opt/trn_rl_repo")

H, D = 4, 128          # heads, head dim
N, F = 4096, 512       # nodes, features
M = 8                  # cores
NP = N // M            # 512 node rows per core
JB = N // 128          # 32 j blocks
IB = NP // 128         # 4 i blocks
KB = F // 128          # 4 contraction blocks
ALPHA = 0.2

_CACHE = {}


def _build_nc():
    import concourse.bacc as bacc
    from concourse import mybir
    from concourse.tile import TileContext

    f32 = mybir.dt.float32
    bf16 = mybir.dt.bfloat16
    Alu = mybir.AluOpType
    Act = mybir.ActivationFunctionType

    nc = bacc.Bacc()
    xT_d = nc.declare_dram_parameter("xT", [F, NP], f32, isOutput=False)
    W_d = nc.declare_dram_parameter("W", [F, F], f32, isOutput=False)
    WA_d = nc.declare_dram_parameter("WA", [F, 2 * H], f32, isOutput=False)
    maskT_d = nc.declare_dram_parameter("maskT", [N, NP], bf16, isOutput=False)
    sel_d = nc.declare_dram_parameter("sel", [2 * H, H * 128], f32, isOutput=False)
    out_d = nc.declare_dram_parameter("out", [NP, F], f32, isOutput=True)

    with TileContext(nc) as tc:
        with (
            tc.tile_pool(name="const", bufs=1) as const_pool,
            tc.tile_pool(name="dram", bufs=1, space="DRAM") as dram_pool,
        ):
            cc_in = dram_pool.tile([NP, F + 2 * H], f32)
            cc_out = dram_pool.tile([N, F + 2 * H], f32, addr_space="Shared")

            # ---- load inputs ----
            xT_sb = const_pool.tile([128, KB * NP], f32)     # k-tiles side by side
            W_sb = const_pool.tile([128, KB * F], f32)
            WA_sb = const_pool.tile([128, KB * 2 * H], f32)
            for k in range(KB):
                nc.sync.dma_start(xT_sb[:, k * NP:(k + 1) * NP],
                                  xT_d[k * 128:(k + 1) * 128, :])
                nc.sync.dma_start(W_sb[:, k * F:(k + 1) * F],
                                  W_d[k * 128:(k + 1) * 128, :])
                nc.sync.dma_start(WA_sb[:, k * 2 * H:(k + 1) * 2 * H],
                                  WA_d[k * 128:(k + 1) * 128, :])

            # one-hot selector rows: sel[k, h*128+m] = (k == h), used to
            # broadcast row h of sT_sb across all 128 output partitions
            sel = const_pool.tile([2 * H, H * 128], f32)
            nc.sync.dma_start(sel[:], sel_d[:])

            # ---- stage A: h_own, S_own, S_T_own ----
            sT_sb = const_pool.tile([8, NP], f32)            # S_T rows 0..3 si, 4..7 sj
            with (
                tc.tile_pool(name="hpsum", bufs=2, space="PSUM") as hpsum,
                tc.tile_pool(name="hstage", bufs=2) as hstage,
            ):
                for ib in range(IB):
                    ph = hpsum.tile([128, F], f32, tag="ph")
                    for k in range(KB):
                        nc.tensor.matmul(
                            ph[:],
                            lhsT=xT_sb[:, k * NP + ib * 128: k * NP + (ib + 1) * 128],
                            rhs=W_sb[:, k * F:(k + 1) * F],
                            start=(k == 0), stop=(k == KB - 1))
                    hsb = hstage.tile([128, F], f32, tag="hsb")
                    nc.scalar.activation(hsb[:], ph[:], Act.Copy)
                    nc.sync.dma_start(cc_in[ib * 128:(ib + 1) * 128, 0:F], hsb[:])

                    ps = hpsum.tile([128, 2 * H], f32, tag="ps")
                    for k in range(KB):
                        nc.tensor.matmul(
                            ps[:],
                            lhsT=xT_sb[:, k * NP + ib * 128: k * NP + (ib + 1) * 128],
                            rhs=WA_sb[:, k * 2 * H:(k + 1) * 2 * H],
                            start=(k == 0), stop=(k == KB - 1))
                    ssb = hstage.tile([128, 2 * H], f32, tag="ssb")
                    nc.vector.tensor_copy(ssb[:], ps[:])
                    nc.sync.dma_start(cc_in[ib * 128:(ib + 1) * 128, F:F + 2 * H], ssb[:])

                pst = hpsum.tile([8, NP], f32, tag="pst")
                for k in range(KB):
                    nc.tensor.matmul(
                        pst[:],
                        lhsT=WA_sb[:, k * 2 * H:(k + 1) * 2 * H],
                        rhs=xT_sb[:, k * NP:(k + 1) * NP],
                        start=(k == 0), stop=(k == KB - 1))
                nc.vector.tensor_copy(sT_sb[:], pst[:])

            # ---- stage B: AllGather [h | S] ----
            nc.gpsimd.collective_compute(
                "AllGather",
                mybir.AluOpType.bypass,
                replica_groups=[list(range(M))],
                ins=[cc_in.opt()],
                outs=[cc_out.opt()],
            )

            # ---- si broadcast tiles + exp(si/5) ----
            si_b = const_pool.tile([128, H * NP], f32)    # exp arg input per head
            E2b = const_pool.tile([128, H * NP], bf16)    # exp(si/5) per head
            with tc.tile_pool(name="bpsum", bufs=2, space="PSUM") as bpsum:
                for h in range(H):
                    pb = bpsum.tile([128, NP], f32, tag="pb")
                    nc.tensor.matmul(pb[:], lhsT=sel[:, h * 128:(h + 1) * 128],
                                     rhs=sT_sb[:], start=True, stop=True)
                    nc.scalar.activation(si_b[:, h * NP:(h + 1) * NP], pb[:], Act.Copy)
                    nc.scalar.activation(E2b[:, h * NP:(h + 1) * NP], pb[:],
                                         Act.Exp, scale=ALPHA)

            # ---- main attention loop ----
            with (
                tc.tile_pool(name="acc", bufs=1, space="PSUM") as acc_pool,
                tc.tile_pool(name="stream", bufs=3) as stream,
                tc.tile_pool(name="tail", bufs=2) as tail_pool,
            ):
                # accumulators: per i-block, two tiles of [128, 2*129] (heads 0-1, 2-3)
                acc = [[acc_pool.tile([128, 2 * (D + 1)], f32,
                                      name=f"acc_{ib}_{g}") for g in range(2)]
                       for ib in range(IB)]

                for jb in range(JB):
                    hs = stream.tile([128, F + 2 * H], f32, tag="hs")
                    nc.sync.dma_start(hs[:], cc_out[jb * 128:(jb + 1) * 128, :])

                    # h chunk -> bf16 aug layout [h0|1|h1|1|h2|1|h3|1]
                    haug = stream.tile([128, H * (D + 1)], bf16, tag="haug")
                    haug3 = haug.rearrange("p (a c) -> p a c", c=D + 1)
                    nc.vector.tensor_copy(
                        haug3[:, :, 0:D],
                        hs[:, 0:F].rearrange("p (a c) -> p a c", c=D))
                    nc.vector.memset(haug3[:, :, D:D + 1], 1.0)

                    # exp(sj/5) for the 4 heads
                    f2 = stream.tile([128, H], f32, tag="f2")
                    nc.scalar.activation(f2[:], hs[:, F + H:F + 2 * H],
                                         Act.Exp, scale=ALPHA)

                    mask = stream.tile([128, NP], bf16, tag="mask")
                    nc.sync.dma_start(mask[:], maskT_d[jb * 128:(jb + 1) * 128, :])

                    for h in range(H):
                        sj_col = hs[:, F + H + h:F + H + h + 1]
                        t1 = stream.tile([128, NP], bf16, tag=f"t1_{h % 2}")
                        nc.scalar.activation(t1[:], si_b[:, h * NP:(h + 1) * NP],
                                             Act.Exp, bias=sj_col)
                        # p = max(exp(si/5)*exp(sj/5), t1)
                        p = stream.tile([128, NP], bf16, tag=f"p_{h % 2}")
                        nc.vector.scalar_tensor_tensor(
                            p[:], in0=E2b[:, h * NP:(h + 1) * NP],
                            scalar=f2[:, h:h + 1], in1=t1[:],
                            op0=Alu.mult, op1=Alu.max)
                        pm = stream.tile([128, NP], bf16, tag=f"pm_{h % 2}")
                        nc.vector.tensor_tensor(pm[:], p[:], mask[:], op=Alu.mult)

                        g, lh = divmod(h, 2)
                        for ib in range(IB):
                            # start=True clears the whole PSUM bank, so only
                            # the first head (lh==0) in each shared bank may
                            # set it; lh==1's first write lands on cleared
                            # has_written bits and overwrites.
                            nc.tensor.matmul(
                                acc[ib][g][:, lh * (D + 1):(lh + 1) * (D + 1)],
                                lhsT=pm[:, ib * 128:(ib + 1) * 128],
                                rhs=haug3[:, h, :],
                                start=(jb == 0 and lh == 0),
                                stop=(jb == JB - 1 and lh == 1),
                                skip_group_check=True)

                # ---- tail: normalize + elu + store ----
                for ib in range(IB):
                    rinv = tail_pool.tile([128, H], f32, tag="rinv")
                    for h in range(H):
                        g, lh = divmod(h, 2)
                        nc.vector.reciprocal(
                            rinv[:, h:h + 1],
                            acc[ib][g][:, lh * (D + 1) + D: lh * (D + 1) + D + 1])
                    osb = tail_pool.tile([128, F], f32, tag="osb")
                    for h in range(H):
                        g, lh = divmod(h, 2)
                        nc.vector.tensor_scalar(
                            osb[:, h * D:(h + 1) * D],
                            in0=acc[ib][g][:, lh * (D + 1): lh * (D + 1) + D],
                            scalar1=rinv[:, h:h + 1], scalar2=None, op0=Alu.mult)
                    # elu(x) = (relu(x) - 1) + exp(min(x, 0))
                    zmin = tail_pool.tile([128, F], f32, tag="zmin")
                    nc.vector.tensor_scalar(zmin[:], in0=osb[:], scalar1=0.0,
                                            scalar2=None, op0=Alu.min)
                    ez = tail_pool.tile([128, F], f32, tag="ez")
                    nc.scalar.activation(ez[:], zmin[:], Act.Exp)
                    rm1 = tail_pool.tile([128, F], f32, tag="rm1")
                    nc.vector.tensor_scalar(rm1[:], in0=osb[:], scalar1=0.0,
                                            scalar2=-1.0, op0=Alu.max, op1=Alu.add)
                    oo = tail_pool.tile([128, F], f32, tag="oo")
                    nc.vector.tensor_tensor(oo[:], ez[:], rm1[:], op=Alu.add)
                    nc.sync.dma_start(out_d[ib * 128:(ib + 1) * 128, :], oo[:])

    nc.compile()
    return nc


def _host_prep(x, adj, W, a):
    x = np.ascontiguousarray(np.asarray(x, np.float32))
    adj = np.asarray(adj)
    W = np.ascontiguousarray(np.asarray(W, np.float32))
    a = np.asarray(a, np.float32)

    A = np.zeros((F, 2 * H), np.float32)
    for h in range(H):
        A[h * D:(h + 1) * D, h] = a[:D, 0]
        A[h * D:(h + 1) * D, H + h] = a[D:, 0]
    WA = np.ascontiguousarray(W @ A)

    import ml_dtypes
    xT = np.ascontiguousarray(x.T)
    adjT = np.ascontiguousarray(adj.T.astype(ml_dtypes.bfloat16))
    sel = np.zeros((2 * H, H * 128), np.float32)
    for h in range(H):
        sel[h, h * 128:(h + 1) * 128] = 1.0

    in_maps = []
    for c in range(M):
        cols = slice(c * NP, (c + 1) * NP)
        in_maps.append({
            "xT": np.ascontiguousarray(xT[:, cols]),
            "W": W,
            "WA": WA,
            "maskT": np.ascontiguousarray(adjT[:, cols]),
            "sel": sel,
        })
    return in_maps


def kernel(x, adj, W, a):
    from concourse.bass_utils import run_bass_kernel_spmd

    if "nc" not in _CACHE:
        _CACHE["nc"] = _build_nc()
    nc = _CACHE["nc"]

    in_maps = _host_prep(x, adj, W, a)
    res = run_bass_kernel_spmd(nc, in_maps, list(range(M)))
    outs = [np.asarray(r["out"], np.float32) for r in res.results]
    return np.concatenate(outs, axis=0)


if __name__ == "__main__":
    nc = _build_nc()
    print("built ok")

